# revision 16
# baseline (speedup 1.0000x reference)
"""Trainium2 Bass kernel for nn_DiseaseKnowledgeModule.

Reference computation (per batch b):
    z_pooled = mean(z_fused[b], axis=S)                      # [D]
    scores   = z_pooled @ M[n,s,:] / sqrt(D)                 # [14, 2]
    alpha    = softmax(scores, axis=-1)
    mlc[b]   = alpha[:, 1]                                   # sigmoid(s1-s0)
    ah       = (mlc[b] > 0.2)                                # {0,1}
    R        = ah @ M[:, 1, :]                               # [D]
    z_out[b] = z_fused[b] + R                                # broadcast over S

Sharding: data-parallel over batch, 2 batches per core on 8 cores; M
replicated.  Memory-bound: per core 32 MB in + 32 MB out.  Each batch
(16 MB) is kept SBUF-resident between the pooling pass and the
broadcast-add so z is read exactly once.

Per core, z[b] is streamed as 8 chunks of [128(S) x 4096] (2 MiB HWDGE
DMAs) into a 10-slot resident ring.  Pooling runs on PE as bf16
ones-matmuls (ScalarE makes a bf16 shadow copy; only the
pooled->softmax path sees bf16 — z_out stays exact f32).  The
threshold compares logits directly (sigmoid is monotone), so the mlc
sigmoid forks off the critical chain.  R is broadcast to 128 partitions
with an exact fp32 ones-matmul; DVE adds it in place; chunks stream
back out.  Emission interleaves batch 1 loads between batch 0 stores
(same sync-engine FIFO) and defers the last two batch-0 stores to fill
batch 1's reduction bubble; tiny DMAs (M, mlc) ride the gpsimd queue.
"""

import numpy as np

B, S, D = 16, 4096, 1024
ND, NS = 14, 2
THRESH = 0.2
LOGIT_THRESH = float(np.log(THRESH / (1.0 - THRESH)))
NCORES = 8
BPC = B // NCORES          # batches per core
NCHUNK = 8                 # chunks per batch
CHUNK_Q = 4                # 128-row subtiles per chunk
CHUNK_F = CHUNK_Q * D      # free elements per chunk (4096)
NSLOT = 11                 # resident ring slots (11 x 2 MiB = 22 MB SBUF)
SCALE = 1.0 / (S * float(np.sqrt(D)))  # fold mean and 1/sqrt(D): 2**-17

_CACHE = {}
LAST_RESULTS = None


def _build_nc():
    import concourse.bacc as bacc
    import concourse.mybir as mybir
    import concourse.tile as tile
    from concourse.masks import make_identity

    f32 = mybir.dt.float32
    bf16 = mybir.dt.bfloat16

    nc = bacc.Bacc("TRN2", target_bir_lowering=False,
                   dynamic_dma_scratch_size=4096)
    z = nc.dram_tensor("z", [BPC, S, D], f32, kind="ExternalInput")
    m = nc.dram_tensor("m", [ND, NS, D], f32, kind="ExternalInput")
    z_out = nc.dram_tensor("z_out", [BPC, S, D], f32, kind="ExternalOutput")
    mlc_out = nc.dram_tensor("mlc", [BPC, ND], f32, kind="ExternalOutput")

    with tile.TileContext(nc) as tc:
        with (
            tc.tile_pool(name="const", bufs=1) as const,
            tc.tile_pool(name="res", bufs=1) as respool,
            tc.tile_pool(name="small", bufs=1) as small,
            tc.tile_pool(name="bfp", bufs=2) as bfp,
            tc.tile_pool(name="rbc", bufs=2) as rbcpool,
            tc.tile_pool(name="ppsum", bufs=2, space="PSUM") as ppsum,
            tc.tile_pool(name="bpsum", bufs=1, space="PSUM") as bpsum,
            tc.tile_pool(name="spsum", bufs=1, space="PSUM") as spsum,
        ):
            # ---- constants ----
            ones_k = const.tile([128, 1], bf16, tag="ones_k")
            nc.vector.memset(ones_k[:], 1.0)
            one_1 = const.tile([1, 1], bf16, tag="one_1")
            nc.vector.memset(one_1[:], 1.0)
            ones_r = const.tile([1, 128], f32, tag="ones_r")
            nc.vector.memset(ones_r[:], 1.0)
            ident = const.tile([2 * ND, 2 * ND], bf16, tag="ident")
            make_identity(nc, ident[:])

            m_nat = const.tile([2 * ND, D], bf16, tag="m_nat")
            nc.gpsimd.dma_start(out=m_nat[:], in_=m[:].rearrange("n s d -> (n s) d"))
            m_pres = const.tile([ND, D], f32, tag="m_pres")
            nc.gpsimd.dma_start(out=m_pres[:], in_=m[:, 1, :])

            # M^T chunks: m_t[:, 28c:28c+28][p, ns] = M_flat[ns, 128c+p]
            m_t = const.tile([128, 8 * 2 * ND], bf16, tag="m_t")
            for c in range(8):
                tr_ps = spsum.tile([128, 2 * ND], bf16, tag="setup")
                nc.tensor.transpose(
                    tr_ps[:], m_nat[:, c * 128 : (c + 1) * 128], ident[:]
                )
                nc.vector.tensor_copy(
                    out=m_t[:, c * 2 * ND : (c + 1) * 2 * ND], in_=tr_ps[:]
                )

            # ---- resident ring ----
            res = respool.tile([128, NSLOT * CHUNK_F], f32, tag="res")

            z_v = z[:].rearrange("b (i q p) d -> b i p q d", p=128, q=CHUNK_Q)
            zo_v = z_out[:].rearrange("b (i q p) d -> b i p q d", p=128, q=CHUNK_Q)

            paccs = {}

            def slot_of(b, i):
                return (b * NCHUNK + i) % NSLOT

            def sl_of(b, i):
                s0 = slot_of(b, i) * CHUNK_F
                return res[:, s0 : s0 + CHUNK_F]

            def load_chunk(b, i):
                """in-DMA + bf16 shadow + pooling matmuls for chunk i of batch b.

                Chunks 0-3 accumulate into pacc_a, 4-7 into pacc_b; scores are
                linear in the pool sum, so the first half's score contribution
                is flushed mid-stream (partial_scores) and the end-of-batch
                critical chain only covers the second half.
                """
                sl = sl_of(b, i)
                nc.sync.dma_start(
                    out=sl.rearrange("p (q d) -> p q d", q=CHUNK_Q),
                    in_=z_v[b, i],
                )
                pacc = paccs[(b, i // 4)]
                first, last = i % 4 == 0, i % 4 == 3
                for half in range(2):
                    bft = bfp.tile([128, CHUNK_F // 2], bf16, tag="bfs",
                                   name="bft")
                    nc.scalar.activation(
                        out=bft[:],
                        in_=sl[:, half * (CHUNK_F // 2) : (half + 1) * (CHUNK_F // 2)],
                        func=mybir.ActivationFunctionType.Copy,
                    )
                    for jj in range(4):  # 512-wide columns; d-half h = j % 2
                        j = half * 4 + jj
                        h = j % 2
                        nc.tensor.matmul(
                            pacc[:, h * 512 : (h + 1) * 512],
                            ones_k[:],
                            bft[:, jj * 512 : (jj + 1) * 512],
                            start=(first and j < 2),
                            stop=(last and j >= 6),
                        )

            sc_accs = {}

            def partial_scores(b, half, sc_first, sc_last):
                """Flush pacc_(a|b) into the shared scores accumulator."""
                pacc = paccs[(b, half)]
                pooled = small.tile([1, 1024], bf16, tag="pooled")
                nc.scalar.activation(
                    out=pooled[:], in_=pacc[:],
                    func=mybir.ActivationFunctionType.Copy, scale=SCALE,
                )
                pt_ps = bpsum.tile([128, 8], f32, tag="pt")
                for c in range(8):
                    nc.tensor.matmul(
                        pt_ps[:, c : c + 1],
                        pooled[:, c * 128 : (c + 1) * 128],
                        one_1[:], start=True, stop=True,
                    )
                pooled_t = small.tile([128, 8], bf16, tag="pooled_t")
                nc.scalar.copy(out=pooled_t[:], in_=pt_ps[:])

                if sc_first:
                    sc_accs[b] = bpsum.tile(
                        [1, 2 * ND], f32, tag="sc", name=f"sc_ps{b}"
                    )
                sc_ps = sc_accs[b]
                for c in range(8):
                    nc.tensor.matmul(
                        sc_ps[:],
                        pooled_t[:, c : c + 1],
                        m_t[:, c * 2 * ND : (c + 1) * 2 * ND],
                        start=(sc_first and c == 0),
                        stop=(sc_last and c == 7),
                    )

            def phase_b(b):
                """scores -> threshold -> r_bc for batch b (second-half flush
                plus the tiny decision chain).

                Returns (r_bc, diff); the mlc sigmoid/DMA is emitted
                separately (phase_b_mlc) so it doesn't sit in the ACT FIFO
                ahead of the next batch's bf16 shadow copies.
                """
                partial_scores(b, 1, sc_first=False, sc_last=True)
                sc_ps = sc_accs[b]

                sc_sb = small.tile([1, 2 * ND], f32, tag="sc_sb")
                nc.scalar.copy(out=sc_sb[:], in_=sc_ps[:])
                sc_v = sc_sb[:].rearrange("p (n s) -> p n s", s=2)
                diff = small.tile([1, ND], f32, tag="diff")
                nc.vector.tensor_sub(diff[:], sc_v[:, :, 1], sc_v[:, :, 0])

                # sigmoid(x) > t  <=>  x > logit(t); scores are ~1e-4 and the
                # threshold is at logit(0.2) = -1.386, so no tie risk.
                ah = small.tile([1, ND], f32, tag="ah")
                nc.vector.tensor_scalar(
                    out=ah[:], in0=diff[:],
                    scalar1=LOGIT_THRESH, scalar2=None,
                    op0=mybir.AluOpType.is_gt,
                )
                # replicate ah along free: [14, 128] with ah_rep[n, p] = ah[n]
                ar_ps = bpsum.tile([ND, 128], f32, tag="sc")
                nc.tensor.matmul(ar_ps[:], ah[:], ones_r[:], start=True, stop=True)
                ah_rep = small.tile([ND, 128], f32, tag="ah_rep")
                nc.scalar.copy(out=ah_rep[:], in_=ar_ps[:])

                # r_bc[p, d] = sum_n ah[n] * M_pres[n, d] (exact fp32),
                # broadcast across partitions in the same matmul.
                r_bc = rbcpool.tile([128, 1024], f32, tag="r_bc")
                for hh in range(2):
                    bc_ps = bpsum.tile([128, 512], f32, tag="r")
                    nc.tensor.matmul(
                        bc_ps[:], ah_rep[:],
                        m_pres[:, hh * 512 : (hh + 1) * 512],
                        start=True, stop=True,
                    )
                    nc.scalar.copy(
                        out=r_bc[:, hh * 512 : (hh + 1) * 512], in_=bc_ps[:]
                    )
                return r_bc, diff

            mlc_tiles = {}

            def phase_b_mlc(b, diff):
                mlc_sb = small.tile([1, ND], f32, tag=f"mlc_sb{b}",
                                    name=f"mlc_sb{b}")
                nc.scalar.activation(
                    out=mlc_sb[:], in_=diff[:],
                    func=mybir.ActivationFunctionType.Sigmoid,
                )
                mlc_tiles[b] = mlc_sb

            def add_chunk(b, i, r_bc):
                sl = sl_of(b, i)
                for q in range(CHUNK_Q):
                    nc.vector.tensor_add(
                        sl[:, q * D : (q + 1) * D],
                        sl[:, q * D : (q + 1) * D],
                        r_bc[:],
                    )

            def store_chunk(b, i, split=False):
                # writes ride the gpsimd (SWDGE) queue: they are paced by the
                # DVE adds, and on their own queue a stalled write never
                # blocks the free-running read stream on the sync queue.
                sl3 = sl_of(b, i).rearrange("p (q d) -> p q d", q=CHUNK_Q)
                if split:
                    # first chunk after the reduction: store per 512KB q-piece
                    # right behind its add so the write stream starts sooner
                    for q in range(CHUNK_Q):
                        nc.gpsimd.dma_start(out=zo_v[b, i][:, q], in_=sl3[:, q])
                else:
                    nc.gpsimd.dma_start(out=zo_v[b, i], in_=sl3)

            # ---- emission schedule ----
            # Per-engine FIFOs follow emission order, so each engine's queue
            # is laid out deliberately: the sync queue runs pure-direction
            # stream blocks; phase_b's PE chain is emitted before batch 1's
            # pooling matmuls so the reduction isn't stuck behind them.
            paccs[(0, 0)] = ppsum.tile([1, 1024], f32, tag="pacc", name="pacc0a")
            paccs[(0, 1)] = ppsum.tile([1, 1024], f32, tag="pacc", name="pacc0b")
            for i in range(4):
                load_chunk(0, i)
            partial_scores(0, 0, sc_first=True, sc_last=False)
            for i in range(4, NCHUNK):
                load_chunk(0, i)

            r_bc0, diff0 = phase_b(0)

            # batch 1 chunks 0,1 go to the two spare slots; their loads extend
            # batch 0's read stream and overlap its reduction chain.
            paccs[(1, 0)] = ppsum.tile([1, 1024], f32, tag="pacc", name="pacc1a")
            paccs[(1, 1)] = ppsum.tile([1, 1024], f32, tag="pacc", name="pacc1b")
            load_chunk(1, 0)
            load_chunk(1, 1)
            load_chunk(1, 2)
            phase_b_mlc(0, diff0)

            # drain batch 0 (write stream), then the rest of batch 1's loads
            # (read stream); in(1, k+3) reuses the slot freed by out(0, k).
            for k in range(NCHUNK):
                add_chunk(0, k, r_bc0)
                if k < 5:
                    store_chunk(0, k, split=(k == 0))
            load_chunk(1, 3)
            partial_scores(1, 0, sc_first=True, sc_last=False)
            for k in range(4, NCHUNK):
                load_chunk(1, k)
            # the last three stores are deferred: batch 1 doesn't reuse their
            # slots, and at runtime they fill batch 1's reduction bubble.
            store_chunk(0, 5)
            store_chunk(0, 6)
            store_chunk(0, 7)

            r_bc1, diff1 = phase_b(1)
            phase_b_mlc(1, diff1)
            for k in range(NCHUNK):
                add_chunk(1, k, r_bc1)
                store_chunk(1, k, split=(k == 0))
            for b in range(BPC):
                nc.sync.dma_start(out=mlc_out[b : b + 1, :], in_=mlc_tiles[b][:])

    nc.finalize()
    return nc


def _get_nc():
    if "nc" not in _CACHE:
        _CACHE["nc"] = _build_nc()
    return _CACHE["nc"]


def kernel(z_fused, M):
    import os

    from concourse.bass_utils import run_bass_kernel_spmd

    global LAST_RESULTS
    nc = _get_nc()
    z_fused = np.ascontiguousarray(z_fused, dtype=np.float32)
    M = np.ascontiguousarray(M, dtype=np.float32)
    in_maps = [
        {"z": z_fused[c * BPC : (c + 1) * BPC], "m": M} for c in range(NCORES)
    ]
    kwargs = {}
    if os.environ.get("KERNEL_TRACE"):
        kwargs["trace"] = True
        if os.environ.get("KERNEL_TMPDIR"):
            kwargs["tmpdir"] = os.environ["KERNEL_TMPDIR"]
    res = run_bass_kernel_spmd(nc, in_maps, list(range(NCORES)), **kwargs)
    LAST_RESULTS = res
    z_out = np.concatenate(
        [res.results[c]["z_out"] for c in range(NCORES)], axis=0
    )
    mlc = np.concatenate([res.results[c]["mlc"] for c in range(NCORES)], axis=0)
    return z_out, mlc


# revision 17
# speedup vs baseline: 1.0405x; 1.0405x over previous
"""Trainium2 Bass kernel for nn_DiseaseKnowledgeModule.

Reference computation (per batch b):
    z_pooled = mean(z_fused[b], axis=S)                      # [D]
    scores   = z_pooled @ M[n,s,:] / sqrt(D)                 # [14, 2]
    alpha    = softmax(scores, axis=-1)
    mlc[b]   = alpha[:, 1]                                   # sigmoid(s1-s0)
    ah       = (mlc[b] > 0.2)                                # {0,1}
    R        = ah @ M[:, 1, :]                               # [D]
    z_out[b] = z_fused[b] + R                                # broadcast over S

Sharding: data-parallel over batch, 2 batches per core on 8 cores; M
replicated.  Memory-bound: per core 32 MB in + 32 MB out.  Each batch
(16 MB) is kept SBUF-resident between the pooling pass and the
broadcast-add so z is read exactly once.

Per core, z[b] is streamed as 8 chunks of [128(S) x 4096] (2 MiB HWDGE
DMAs) into a 10-slot resident ring.  Pooling runs on PE as bf16
ones-matmuls (ScalarE makes a bf16 shadow copy; only the
pooled->softmax path sees bf16 — z_out stays exact f32).  The
threshold compares logits directly (sigmoid is monotone), so the mlc
sigmoid forks off the critical chain.  R is broadcast to 128 partitions
with an exact fp32 ones-matmul; DVE adds it in place; chunks stream
back out.  Emission interleaves batch 1 loads between batch 0 stores
(same sync-engine FIFO) and defers the last two batch-0 stores to fill
batch 1's reduction bubble; tiny DMAs (M, mlc) ride the gpsimd queue.
"""

import numpy as np

B, S, D = 16, 4096, 1024
ND, NS = 14, 2
THRESH = 0.2
LOGIT_THRESH = float(np.log(THRESH / (1.0 - THRESH)))
NCORES = 8
BPC = B // NCORES          # batches per core
NCHUNK = 8                 # chunks per batch
CHUNK_Q = 4                # 128-row subtiles per chunk
CHUNK_F = CHUNK_Q * D      # free elements per chunk (4096)
NSLOT = 11                 # resident ring slots (11 x 2 MiB = 22 MB SBUF)
SCALE = 1.0 / (S * float(np.sqrt(D)))  # fold mean and 1/sqrt(D): 2**-17

_CACHE = {}
LAST_RESULTS = None


def _build_nc():
    import concourse.bacc as bacc
    import concourse.mybir as mybir
    import concourse.tile as tile
    from concourse.masks import make_identity

    f32 = mybir.dt.float32
    bf16 = mybir.dt.bfloat16

    nc = bacc.Bacc("TRN2", target_bir_lowering=False,
                   dynamic_dma_scratch_size=4096)
    z = nc.dram_tensor("z", [BPC, S, D], f32, kind="ExternalInput")
    m = nc.dram_tensor("m", [ND, NS, D], f32, kind="ExternalInput")
    z_out = nc.dram_tensor("z_out", [BPC, S, D], f32, kind="ExternalOutput")
    mlc_out = nc.dram_tensor("mlc", [BPC, ND], f32, kind="ExternalOutput")

    with tile.TileContext(nc) as tc:
        with (
            tc.tile_pool(name="const", bufs=1) as const,
            tc.tile_pool(name="res", bufs=1) as respool,
            tc.tile_pool(name="small", bufs=1) as small,
            tc.tile_pool(name="bfp", bufs=2) as bfp,
            tc.tile_pool(name="rbc", bufs=2) as rbcpool,
            tc.tile_pool(name="ppsum", bufs=2, space="PSUM") as ppsum,
            tc.tile_pool(name="bpsum", bufs=1, space="PSUM") as bpsum,
            tc.tile_pool(name="spsum", bufs=1, space="PSUM") as spsum,
        ):
            # ---- constants ----
            ones_k = const.tile([128, 1], bf16, tag="ones_k")
            nc.vector.memset(ones_k[:], 1.0)
            one_1 = const.tile([1, 1], bf16, tag="one_1")
            nc.vector.memset(one_1[:], 1.0)
            ones_r = const.tile([1, 128], f32, tag="ones_r")
            nc.vector.memset(ones_r[:], 1.0)
            ident = const.tile([2 * ND, 2 * ND], bf16, tag="ident")
            make_identity(nc, ident[:])

            m_nat = const.tile([2 * ND, D], bf16, tag="m_nat")
            nc.gpsimd.dma_start(out=m_nat[:], in_=m[:].rearrange("n s d -> (n s) d"))
            m_pres = const.tile([ND, D], f32, tag="m_pres")
            nc.gpsimd.dma_start(out=m_pres[:], in_=m[:, 1, :])

            # M^T chunks: m_t[:, 28c:28c+28][p, ns] = M_flat[ns, 128c+p]
            m_t = const.tile([128, 8 * 2 * ND], bf16, tag="m_t")
            for c in range(8):
                tr_ps = spsum.tile([128, 2 * ND], bf16, tag="setup")
                nc.tensor.transpose(
                    tr_ps[:], m_nat[:, c * 128 : (c + 1) * 128], ident[:]
                )
                nc.vector.tensor_copy(
                    out=m_t[:, c * 2 * ND : (c + 1) * 2 * ND], in_=tr_ps[:]
                )

            # ---- resident ring ----
            res = respool.tile([128, NSLOT * CHUNK_F], f32, tag="res")

            z_v = z[:].rearrange("b (i q p) d -> b i p q d", p=128, q=CHUNK_Q)
            zo_v = z_out[:].rearrange("b (i q p) d -> b i p q d", p=128, q=CHUNK_Q)

            paccs = {}

            def slot_of(b, i):
                return (b * NCHUNK + i) % NSLOT

            def sl_of(b, i):
                s0 = slot_of(b, i) * CHUNK_F
                return res[:, s0 : s0 + CHUNK_F]

            def load_chunk(b, i):
                """in-DMA + bf16 shadow + pooling matmuls for chunk i of batch b.

                Chunks 0-3 accumulate into pacc_a, 4-7 into pacc_b; scores are
                linear in the pool sum, so the first half's score contribution
                is flushed mid-stream (partial_scores) and the end-of-batch
                critical chain only covers the second half.
                """
                sl = sl_of(b, i)
                nc.sync.dma_start(
                    out=sl.rearrange("p (q d) -> p q d", q=CHUNK_Q),
                    in_=z_v[b, i],
                )
                pacc = paccs[(b, i // 4)]
                first, last = i % 4 == 0, i % 4 == 3
                for half in range(2):
                    bft = bfp.tile([128, CHUNK_F // 2], bf16, tag="bfs",
                                   name="bft")
                    nc.scalar.activation(
                        out=bft[:],
                        in_=sl[:, half * (CHUNK_F // 2) : (half + 1) * (CHUNK_F // 2)],
                        func=mybir.ActivationFunctionType.Copy,
                    )
                    for jj in range(4):  # 512-wide columns; d-half h = j % 2
                        j = half * 4 + jj
                        h = j % 2
                        nc.tensor.matmul(
                            pacc[:, h * 512 : (h + 1) * 512],
                            ones_k[:],
                            bft[:, jj * 512 : (jj + 1) * 512],
                            start=(first and j < 2),
                            stop=(last and j >= 6),
                        )

            sc_accs = {}

            def partial_scores(b, half, sc_first, sc_last):
                """Flush pacc_(a|b) into the shared scores accumulator."""
                pacc = paccs[(b, half)]
                pooled = small.tile([1, 1024], bf16, tag="pooled")
                nc.scalar.activation(
                    out=pooled[:], in_=pacc[:],
                    func=mybir.ActivationFunctionType.Copy, scale=SCALE,
                )
                pt_ps = bpsum.tile([128, 8], f32, tag="pt")
                for c in range(8):
                    nc.tensor.matmul(
                        pt_ps[:, c : c + 1],
                        pooled[:, c * 128 : (c + 1) * 128],
                        one_1[:], start=True, stop=True,
                    )
                pooled_t = small.tile([128, 8], bf16, tag="pooled_t")
                nc.scalar.copy(out=pooled_t[:], in_=pt_ps[:])

                if sc_first:
                    sc_accs[b] = bpsum.tile(
                        [1, 2 * ND], f32, tag="sc", name=f"sc_ps{b}"
                    )
                sc_ps = sc_accs[b]
                for c in range(8):
                    nc.tensor.matmul(
                        sc_ps[:],
                        pooled_t[:, c : c + 1],
                        m_t[:, c * 2 * ND : (c + 1) * 2 * ND],
                        start=(sc_first and c == 0),
                        stop=(sc_last and c == 7),
                    )

            def phase_b(b):
                """scores -> threshold -> r_bc for batch b (second-half flush
                plus the tiny decision chain).

                Returns (r_bc, diff); the mlc sigmoid/DMA is emitted
                separately (phase_b_mlc) so it doesn't sit in the ACT FIFO
                ahead of the next batch's bf16 shadow copies.
                """
                partial_scores(b, 1, sc_first=False, sc_last=True)
                sc_ps = sc_accs[b]

                sc_sb = small.tile([1, 2 * ND], f32, tag="sc_sb")
                nc.scalar.copy(out=sc_sb[:], in_=sc_ps[:])
                sc_v = sc_sb[:].rearrange("p (n s) -> p n s", s=2)
                diff = small.tile([1, ND], f32, tag="diff")
                nc.vector.tensor_sub(diff[:], sc_v[:, :, 1], sc_v[:, :, 0])

                # sigmoid(x) > t  <=>  x > logit(t); scores are ~1e-4 and the
                # threshold is at logit(0.2) = -1.386, so no tie risk.
                ah = small.tile([1, ND], f32, tag="ah")
                nc.vector.tensor_scalar(
                    out=ah[:], in0=diff[:],
                    scalar1=LOGIT_THRESH, scalar2=None,
                    op0=mybir.AluOpType.is_gt,
                )
                # replicate ah along free: [14, 128] with ah_rep[n, p] = ah[n]
                ar_ps = bpsum.tile([ND, 128], f32, tag="sc")
                nc.tensor.matmul(ar_ps[:], ah[:], ones_r[:], start=True, stop=True)
                ah_rep = small.tile([ND, 128], f32, tag="ah_rep")
                nc.scalar.copy(out=ah_rep[:], in_=ar_ps[:])

                # r_bc[p, d] = sum_n ah[n] * M_pres[n, d] (exact fp32),
                # broadcast across partitions in the same matmul.
                r_bc = rbcpool.tile([128, 1024], f32, tag="r_bc")
                for hh in range(2):
                    bc_ps = bpsum.tile([128, 512], f32, tag="r")
                    nc.tensor.matmul(
                        bc_ps[:], ah_rep[:],
                        m_pres[:, hh * 512 : (hh + 1) * 512],
                        start=True, stop=True,
                    )
                    nc.scalar.copy(
                        out=r_bc[:, hh * 512 : (hh + 1) * 512], in_=bc_ps[:]
                    )
                return r_bc, diff

            mlc_tiles = {}

            def phase_b_mlc(b, diff):
                mlc_sb = small.tile([1, ND], f32, tag=f"mlc_sb{b}",
                                    name=f"mlc_sb{b}")
                nc.scalar.activation(
                    out=mlc_sb[:], in_=diff[:],
                    func=mybir.ActivationFunctionType.Sigmoid,
                )
                mlc_tiles[b] = mlc_sb

            def add_chunk(b, i, r_bc):
                sl = sl_of(b, i)
                for q in range(CHUNK_Q):
                    nc.vector.tensor_add(
                        sl[:, q * D : (q + 1) * D],
                        sl[:, q * D : (q + 1) * D],
                        r_bc[:],
                    )

            def store_chunk(b, i):
                nc.sync.dma_start(
                    out=zo_v[b, i],
                    in_=sl_of(b, i).rearrange("p (q d) -> p q d", q=CHUNK_Q),
                )

            # ---- emission schedule ----
            # Per-engine FIFOs follow emission order, so each engine's queue
            # is laid out deliberately: the sync queue runs pure-direction
            # stream blocks; phase_b's PE chain is emitted before batch 1's
            # pooling matmuls so the reduction isn't stuck behind them.
            paccs[(0, 0)] = ppsum.tile([1, 1024], f32, tag="pacc", name="pacc0a")
            paccs[(0, 1)] = ppsum.tile([1, 1024], f32, tag="pacc", name="pacc0b")
            for i in range(4):
                load_chunk(0, i)
            partial_scores(0, 0, sc_first=True, sc_last=False)
            for i in range(4, NCHUNK):
                load_chunk(0, i)

            r_bc0, diff0 = phase_b(0)

            # batch 1 chunks 0,1 go to the two spare slots; their loads extend
            # batch 0's read stream and overlap its reduction chain.
            paccs[(1, 0)] = ppsum.tile([1, 1024], f32, tag="pacc", name="pacc1a")
            paccs[(1, 1)] = ppsum.tile([1, 1024], f32, tag="pacc", name="pacc1b")
            load_chunk(1, 0)
            load_chunk(1, 1)
            load_chunk(1, 2)
            phase_b_mlc(0, diff0)

            # drain batch 0 (write stream), then the rest of batch 1's loads
            # (read stream); in(1, k+3) reuses the slot freed by out(0, k).
            for k in range(NCHUNK):
                add_chunk(0, k, r_bc0)
                if k < 5:
                    store_chunk(0, k)
            load_chunk(1, 3)
            partial_scores(1, 0, sc_first=True, sc_last=False)
            for k in range(4, NCHUNK):
                load_chunk(1, k)
            # the last three stores are deferred: batch 1 doesn't reuse their
            # slots, and at runtime they fill batch 1's reduction bubble.
            store_chunk(0, 5)
            store_chunk(0, 6)
            store_chunk(0, 7)

            r_bc1, diff1 = phase_b(1)
            phase_b_mlc(1, diff1)
            for k in range(NCHUNK):
                add_chunk(1, k, r_bc1)
                store_chunk(1, k)
            for b in range(BPC):
                nc.sync.dma_start(out=mlc_out[b : b + 1, :], in_=mlc_tiles[b][:])

    nc.finalize()
    return nc


def _get_nc():
    if "nc" not in _CACHE:
        _CACHE["nc"] = _build_nc()
    return _CACHE["nc"]


def kernel(z_fused, M):
    import os

    from concourse.bass_utils import run_bass_kernel_spmd

    global LAST_RESULTS
    nc = _get_nc()
    z_fused = np.ascontiguousarray(z_fused, dtype=np.float32)
    M = np.ascontiguousarray(M, dtype=np.float32)
    in_maps = [
        {"z": z_fused[c * BPC : (c + 1) * BPC], "m": M} for c in range(NCORES)
    ]
    kwargs = {}
    if os.environ.get("KERNEL_TRACE"):
        kwargs["trace"] = True
        if os.environ.get("KERNEL_TMPDIR"):
            kwargs["tmpdir"] = os.environ["KERNEL_TMPDIR"]
    res = run_bass_kernel_spmd(nc, in_maps, list(range(NCORES)), **kwargs)
    LAST_RESULTS = res
    z_out = np.concatenate(
        [res.results[c]["z_out"] for c in range(NCORES)], axis=0
    )
    mlc = np.concatenate([res.results[c]["mlc"] for c in range(NCORES)], axis=0)
    return z_out, mlc


# revision 18
# speedup vs baseline: 1.0454x; 1.0047x over previous
"""Trainium2 Bass kernel for nn_DiseaseKnowledgeModule.

Reference computation (per batch b):
    z_pooled = mean(z_fused[b], axis=S)                      # [D]
    scores   = z_pooled @ M[n,s,:] / sqrt(D)                 # [14, 2]
    alpha    = softmax(scores, axis=-1)
    mlc[b]   = alpha[:, 1]                                   # sigmoid(s1-s0)
    ah       = (mlc[b] > 0.2)                                # {0,1}
    R        = ah @ M[:, 1, :]                               # [D]
    z_out[b] = z_fused[b] + R                                # broadcast over S

Sharding: data-parallel over batch, 2 batches per core on 8 cores; M
replicated.  Memory-bound: per core 32 MB in + 32 MB out.  Each batch
(16 MB) is kept SBUF-resident between the pooling pass and the
broadcast-add so z is read exactly once.

Per core, z[b] is streamed as 8 chunks of [128(S) x 4096] (2 MiB HWDGE
DMAs) into a 10-slot resident ring.  Pooling runs on PE as bf16
ones-matmuls (ScalarE makes a bf16 shadow copy; only the
pooled->softmax path sees bf16 — z_out stays exact f32).  The
threshold compares logits directly (sigmoid is monotone), so the mlc
sigmoid forks off the critical chain.  R is broadcast to 128 partitions
with an exact fp32 ones-matmul; DVE adds it in place; chunks stream
back out.  Emission interleaves batch 1 loads between batch 0 stores
(same sync-engine FIFO) and defers the last two batch-0 stores to fill
batch 1's reduction bubble; tiny DMAs (M, mlc) ride the gpsimd queue.
"""

import numpy as np

B, S, D = 16, 4096, 1024
ND, NS = 14, 2
THRESH = 0.2
LOGIT_THRESH = float(np.log(THRESH / (1.0 - THRESH)))
NCORES = 8
BPC = B // NCORES          # batches per core
NCHUNK = 8                 # chunks per batch
CHUNK_Q = 4                # 128-row subtiles per chunk
CHUNK_F = CHUNK_Q * D      # free elements per chunk (4096)
NSLOT = 11                 # resident ring slots (11 x 2 MiB = 22 MB SBUF)
SCALE = 1.0 / (S * float(np.sqrt(D)))  # fold mean and 1/sqrt(D): 2**-17

_CACHE = {}
LAST_RESULTS = None


def _build_nc():
    import concourse.bacc as bacc
    import concourse.mybir as mybir
    import concourse.tile as tile
    from concourse.masks import make_identity

    f32 = mybir.dt.float32
    bf16 = mybir.dt.bfloat16

    nc = bacc.Bacc("TRN2", target_bir_lowering=False,
                   dynamic_dma_scratch_size=4096)
    z = nc.dram_tensor("z", [BPC, S, D], f32, kind="ExternalInput")
    m = nc.dram_tensor("m", [ND, NS, D], f32, kind="ExternalInput")
    z_out = nc.dram_tensor("z_out", [BPC, S, D], f32, kind="ExternalOutput")
    mlc_out = nc.dram_tensor("mlc", [BPC, ND], f32, kind="ExternalOutput")

    with tile.TileContext(nc) as tc:
        with (
            tc.tile_pool(name="const", bufs=1) as const,
            tc.tile_pool(name="res", bufs=1) as respool,
            tc.tile_pool(name="small", bufs=1) as small,
            tc.tile_pool(name="bfp", bufs=2) as bfp,
            tc.tile_pool(name="rbc", bufs=2) as rbcpool,
            tc.tile_pool(name="ppsum", bufs=2, space="PSUM") as ppsum,
            tc.tile_pool(name="bpsum", bufs=1, space="PSUM") as bpsum,
            tc.tile_pool(name="spsum", bufs=1, space="PSUM") as spsum,
        ):
            # ---- constants ----
            ones_k = const.tile([128, 1], bf16, tag="ones_k")
            nc.vector.memset(ones_k[:], 1.0)
            one_1 = const.tile([1, 1], bf16, tag="one_1")
            nc.vector.memset(one_1[:], 1.0)
            ones_r = const.tile([1, 128], f32, tag="ones_r")
            nc.vector.memset(ones_r[:], 1.0)
            ident = const.tile([2 * ND, 2 * ND], bf16, tag="ident")
            make_identity(nc, ident[:])

            m_nat = const.tile([2 * ND, D], bf16, tag="m_nat")
            nc.gpsimd.dma_start(out=m_nat[:], in_=m[:].rearrange("n s d -> (n s) d"))
            m_pres = const.tile([ND, D], f32, tag="m_pres")
            nc.gpsimd.dma_start(out=m_pres[:], in_=m[:, 1, :])

            # M^T chunks: m_t[:, 28c:28c+28][p, ns] = M_flat[ns, 128c+p]
            m_t = const.tile([128, 8 * 2 * ND], bf16, tag="m_t")
            for c in range(8):
                tr_ps = spsum.tile([128, 2 * ND], bf16, tag="setup")
                nc.tensor.transpose(
                    tr_ps[:], m_nat[:, c * 128 : (c + 1) * 128], ident[:]
                )
                nc.vector.tensor_copy(
                    out=m_t[:, c * 2 * ND : (c + 1) * 2 * ND], in_=tr_ps[:]
                )

            # ---- resident ring ----
            res = respool.tile([128, NSLOT * CHUNK_F], f32, tag="res")

            z_v = z[:].rearrange("b (i q p) d -> b i p q d", p=128, q=CHUNK_Q)
            zo_v = z_out[:].rearrange("b (i q p) d -> b i p q d", p=128, q=CHUNK_Q)

            paccs = {}

            def slot_of(b, i):
                return (b * NCHUNK + i) % NSLOT

            def sl_of(b, i):
                s0 = slot_of(b, i) * CHUNK_F
                return res[:, s0 : s0 + CHUNK_F]

            def load_chunk(b, i):
                """in-DMA + bf16 shadow + pooling matmuls for chunk i of batch b.

                Chunks 0-3 accumulate into pacc_a, 4-7 into pacc_b; scores are
                linear in the pool sum, so the first half's score contribution
                is flushed mid-stream (partial_scores) and the end-of-batch
                critical chain only covers the second half.
                """
                sl = sl_of(b, i)
                nc.sync.dma_start(
                    out=sl.rearrange("p (q d) -> p q d", q=CHUNK_Q),
                    in_=z_v[b, i],
                )
                pacc = paccs[(b, i // 4)]
                first, last = i % 4 == 0, i % 4 == 3
                for half in range(2):
                    bft = bfp.tile([128, CHUNK_F // 2], bf16, tag="bfs",
                                   name="bft")
                    nc.scalar.activation(
                        out=bft[:],
                        in_=sl[:, half * (CHUNK_F // 2) : (half + 1) * (CHUNK_F // 2)],
                        func=mybir.ActivationFunctionType.Copy,
                    )
                    for jj in range(4):  # 512-wide columns; d-half h = j % 2
                        j = half * 4 + jj
                        h = j % 2
                        nc.tensor.matmul(
                            pacc[:, h * 512 : (h + 1) * 512],
                            ones_k[:],
                            bft[:, jj * 512 : (jj + 1) * 512],
                            start=(first and j < 2),
                            stop=(last and j >= 6),
                        )

            sc_accs = {}

            def partial_scores(b, half, sc_first, sc_last):
                """Flush pacc_(a|b) into the shared scores accumulator."""
                pacc = paccs[(b, half)]
                pooled = small.tile([1, 1024], bf16, tag="pooled")
                nc.scalar.activation(
                    out=pooled[:], in_=pacc[:],
                    func=mybir.ActivationFunctionType.Copy, scale=SCALE,
                )
                pt_ps = bpsum.tile([128, 8], f32, tag="pt")
                for c in range(8):
                    nc.tensor.matmul(
                        pt_ps[:, c : c + 1],
                        pooled[:, c * 128 : (c + 1) * 128],
                        one_1[:], start=True, stop=True,
                    )
                pooled_t = small.tile([128, 8], bf16, tag="pooled_t")
                nc.vector.tensor_copy(out=pooled_t[:], in_=pt_ps[:])

                if sc_first:
                    sc_accs[b] = bpsum.tile(
                        [1, 2 * ND], f32, tag="sc", name=f"sc_ps{b}"
                    )
                sc_ps = sc_accs[b]
                for c in range(8):
                    nc.tensor.matmul(
                        sc_ps[:],
                        pooled_t[:, c : c + 1],
                        m_t[:, c * 2 * ND : (c + 1) * 2 * ND],
                        start=(sc_first and c == 0),
                        stop=(sc_last and c == 7),
                    )

            def phase_b(b):
                """scores -> threshold -> r_bc for batch b (second-half flush
                plus the tiny decision chain).

                Returns (r_bc, diff); the mlc sigmoid/DMA is emitted
                separately (phase_b_mlc) so it doesn't sit in the ACT FIFO
                ahead of the next batch's bf16 shadow copies.
                """
                partial_scores(b, 1, sc_first=False, sc_last=True)
                sc_ps = sc_accs[b]

                sc_sb = small.tile([1, 2 * ND], f32, tag="sc_sb")
                nc.vector.tensor_copy(out=sc_sb[:], in_=sc_ps[:])
                sc_v = sc_sb[:].rearrange("p (n s) -> p n s", s=2)
                diff = small.tile([1, ND], f32, tag="diff")
                nc.vector.tensor_sub(diff[:], sc_v[:, :, 1], sc_v[:, :, 0])

                # sigmoid(x) > t  <=>  x > logit(t); scores are ~1e-4 and the
                # threshold is at logit(0.2) = -1.386, so no tie risk.
                ah = small.tile([1, ND], f32, tag="ah")
                nc.vector.tensor_scalar(
                    out=ah[:], in0=diff[:],
                    scalar1=LOGIT_THRESH, scalar2=None,
                    op0=mybir.AluOpType.is_gt,
                )
                # replicate ah along free: [14, 128] with ah_rep[n, p] = ah[n]
                ar_ps = bpsum.tile([ND, 128], f32, tag="sc")
                nc.tensor.matmul(ar_ps[:], ah[:], ones_r[:], start=True, stop=True)
                ah_rep = small.tile([ND, 128], f32, tag="ah_rep")
                nc.vector.tensor_copy(out=ah_rep[:], in_=ar_ps[:])

                # r_bc[p, d] = sum_n ah[n] * M_pres[n, d] (exact fp32),
                # broadcast across partitions in the same matmul.
                r_bc = rbcpool.tile([128, 1024], f32, tag="r_bc")
                for hh in range(2):
                    bc_ps = bpsum.tile([128, 512], f32, tag="r")
                    nc.tensor.matmul(
                        bc_ps[:], ah_rep[:],
                        m_pres[:, hh * 512 : (hh + 1) * 512],
                        start=True, stop=True,
                    )
                    nc.vector.tensor_copy(
                        out=r_bc[:, hh * 512 : (hh + 1) * 512], in_=bc_ps[:]
                    )
                return r_bc, diff

            mlc_tiles = {}

            def phase_b_mlc(b, diff):
                mlc_sb = small.tile([1, ND], f32, tag=f"mlc_sb{b}",
                                    name=f"mlc_sb{b}")
                nc.scalar.activation(
                    out=mlc_sb[:], in_=diff[:],
                    func=mybir.ActivationFunctionType.Sigmoid,
                )
                mlc_tiles[b] = mlc_sb

            def add_chunk(b, i, r_bc):
                sl = sl_of(b, i)
                for q in range(CHUNK_Q):
                    nc.vector.tensor_add(
                        sl[:, q * D : (q + 1) * D],
                        sl[:, q * D : (q + 1) * D],
                        r_bc[:],
                    )

            def store_chunk(b, i):
                nc.sync.dma_start(
                    out=zo_v[b, i],
                    in_=sl_of(b, i).rearrange("p (q d) -> p q d", q=CHUNK_Q),
                )

            # ---- emission schedule ----
            # Per-engine FIFOs follow emission order, so each engine's queue
            # is laid out deliberately: the sync queue runs pure-direction
            # stream blocks; phase_b's PE chain is emitted before batch 1's
            # pooling matmuls so the reduction isn't stuck behind them.
            paccs[(0, 0)] = ppsum.tile([1, 1024], f32, tag="pacc", name="pacc0a")
            paccs[(0, 1)] = ppsum.tile([1, 1024], f32, tag="pacc", name="pacc0b")
            for i in range(4):
                load_chunk(0, i)
            partial_scores(0, 0, sc_first=True, sc_last=False)
            for i in range(4, NCHUNK):
                load_chunk(0, i)

            r_bc0, diff0 = phase_b(0)

            # batch 1 chunks 0,1 go to the two spare slots; their loads extend
            # batch 0's read stream and overlap its reduction chain.
            paccs[(1, 0)] = ppsum.tile([1, 1024], f32, tag="pacc", name="pacc1a")
            paccs[(1, 1)] = ppsum.tile([1, 1024], f32, tag="pacc", name="pacc1b")
            load_chunk(1, 0)
            load_chunk(1, 1)
            load_chunk(1, 2)
            phase_b_mlc(0, diff0)

            # drain batch 0 (write stream), then the rest of batch 1's loads
            # (read stream); in(1, k+3) reuses the slot freed by out(0, k).
            for k in range(NCHUNK):
                add_chunk(0, k, r_bc0)
                if k < 5:
                    store_chunk(0, k)
            load_chunk(1, 3)
            partial_scores(1, 0, sc_first=True, sc_last=False)
            for k in range(4, NCHUNK):
                load_chunk(1, k)
            # the last three stores are deferred: batch 1 doesn't reuse their
            # slots, and at runtime they fill batch 1's reduction bubble.
            store_chunk(0, 5)
            store_chunk(0, 6)
            store_chunk(0, 7)

            r_bc1, diff1 = phase_b(1)
            phase_b_mlc(1, diff1)
            for k in range(NCHUNK):
                add_chunk(1, k, r_bc1)
                store_chunk(1, k)
            for b in range(BPC):
                nc.sync.dma_start(out=mlc_out[b : b + 1, :], in_=mlc_tiles[b][:])

    nc.finalize()
    return nc


def _get_nc():
    if "nc" not in _CACHE:
        _CACHE["nc"] = _build_nc()
    return _CACHE["nc"]


def kernel(z_fused, M):
    import os

    from concourse.bass_utils import run_bass_kernel_spmd

    global LAST_RESULTS
    nc = _get_nc()
    z_fused = np.ascontiguousarray(z_fused, dtype=np.float32)
    M = np.ascontiguousarray(M, dtype=np.float32)
    in_maps = [
        {"z": z_fused[c * BPC : (c + 1) * BPC], "m": M} for c in range(NCORES)
    ]
    kwargs = {}
    if os.environ.get("KERNEL_TRACE"):
        kwargs["trace"] = True
        if os.environ.get("KERNEL_TMPDIR"):
            kwargs["tmpdir"] = os.environ["KERNEL_TMPDIR"]
    res = run_bass_kernel_spmd(nc, in_maps, list(range(NCORES)), **kwargs)
    LAST_RESULTS = res
    z_out = np.concatenate(
        [res.results[c]["z_out"] for c in range(NCORES)], axis=0
    )
    mlc = np.concatenate([res.results[c]["mlc"] for c in range(NCORES)], axis=0)
    return z_out, mlc


# revision 19
# speedup vs baseline: 1.1782x; 1.1271x over previous
"""Trainium2 Bass kernel for nn_DiseaseKnowledgeModule.

Reference computation (per batch b):
    z_pooled = mean(z_fused[b], axis=S)                      # [D]
    scores   = z_pooled @ M[n,s,:] / sqrt(D)                 # [14, 2]
    alpha    = softmax(scores, axis=-1)
    mlc[b]   = alpha[:, 1]                                   # sigmoid(s1-s0)
    ah       = (mlc[b] > 0.2)                                # {0,1}
    R        = ah @ M[:, 1, :]                               # [D]
    z_out[b] = z_fused[b] + R                                # broadcast over S

Sharding: data-parallel over batch, 2 batches per core on 8 cores; M
replicated.  Memory-bound: per core 32 MB in + 32 MB out.  Each batch
(16 MB) is kept SBUF-resident between the pooling pass and the
broadcast-add so z is read exactly once.

Per core, z[b] is streamed as 8 chunks of [128(S) x 4096] (2 MiB HWDGE
DMAs) into a 10-slot resident ring.  Pooling runs on PE as bf16
ones-matmuls (ScalarE makes a bf16 shadow copy; only the
pooled->softmax path sees bf16 — z_out stays exact f32).  The
threshold compares logits directly (sigmoid is monotone), so the mlc
sigmoid forks off the critical chain.  R is broadcast to 128 partitions
with an exact fp32 ones-matmul; DVE adds it in place; chunks stream
back out.  Emission interleaves batch 1 loads between batch 0 stores
(same sync-engine FIFO) and defers the last two batch-0 stores to fill
batch 1's reduction bubble; tiny DMAs (M, mlc) ride the gpsimd queue.
"""

import numpy as np

B, S, D = 16, 4096, 1024
ND, NS = 14, 2
THRESH = 0.2
LOGIT_THRESH = float(np.log(THRESH / (1.0 - THRESH)))
NCORES = 8
BPC = B // NCORES          # batches per core
NCHUNK = 8                 # chunks per batch
CHUNK_Q = 4                # 128-row subtiles per chunk
CHUNK_F = CHUNK_Q * D      # free elements per chunk (4096)
NSLOT = 11                 # resident ring slots (11 x 2 MiB = 22 MB SBUF)
SCALE = 1.0 / (S * float(np.sqrt(D)))  # fold mean and 1/sqrt(D): 2**-17

_CACHE = {}
LAST_RESULTS = None


def _build_nc():
    import concourse.bacc as bacc
    import concourse.mybir as mybir
    import concourse.tile as tile
    from concourse.masks import make_identity

    f32 = mybir.dt.float32
    bf16 = mybir.dt.bfloat16

    nc = bacc.Bacc("TRN2", target_bir_lowering=False,
                   dynamic_dma_scratch_size=4096)
    z = nc.dram_tensor("z", [BPC, S, D], f32, kind="ExternalInput")
    m = nc.dram_tensor("m", [ND, NS, D], f32, kind="ExternalInput")
    z_out = nc.dram_tensor("z_out", [BPC, S, D], f32, kind="ExternalOutput")
    mlc_out = nc.dram_tensor("mlc", [BPC, ND], f32, kind="ExternalOutput")

    with tile.TileContext(nc) as tc:
        with (
            tc.tile_pool(name="const", bufs=1) as const,
            tc.tile_pool(name="res", bufs=1) as respool,
            tc.tile_pool(name="small", bufs=1) as small,
            tc.tile_pool(name="bfp", bufs=2) as bfp,
            tc.tile_pool(name="rbc", bufs=2) as rbcpool,
            tc.tile_pool(name="ppsum", bufs=2, space="PSUM") as ppsum,
            tc.tile_pool(name="bpsum", bufs=1, space="PSUM") as bpsum,
            tc.tile_pool(name="spsum", bufs=1, space="PSUM") as spsum,
        ):
            # ---- constants ----
            ones_k = const.tile([128, 1], bf16, tag="ones_k")
            nc.vector.memset(ones_k[:], 1.0)
            one_1 = const.tile([1, 1], bf16, tag="one_1")
            nc.vector.memset(one_1[:], 1.0)
            ones_r = const.tile([1, 128], f32, tag="ones_r")
            nc.vector.memset(ones_r[:], 1.0)
            ident = const.tile([2 * ND, 2 * ND], bf16, tag="ident")
            make_identity(nc, ident[:])

            m_nat = const.tile([2 * ND, D], bf16, tag="m_nat")
            nc.gpsimd.dma_start(out=m_nat[:], in_=m[:].rearrange("n s d -> (n s) d"))
            m_pres = const.tile([ND, D], f32, tag="m_pres")
            nc.gpsimd.dma_start(out=m_pres[:], in_=m[:, 1, :])

            # M^T chunks: m_t[:, 28c:28c+28][p, ns] = M_flat[ns, 128c+p]
            m_t = const.tile([128, 8 * 2 * ND], bf16, tag="m_t")
            for c in range(8):
                tr_ps = spsum.tile([128, 2 * ND], bf16, tag="setup")
                nc.tensor.transpose(
                    tr_ps[:], m_nat[:, c * 128 : (c + 1) * 128], ident[:]
                )
                nc.vector.tensor_copy(
                    out=m_t[:, c * 2 * ND : (c + 1) * 2 * ND], in_=tr_ps[:]
                )

            # ---- resident ring ----
            res = respool.tile([128, NSLOT * CHUNK_F], f32, tag="res")

            z_v = z[:].rearrange("b (i q p) d -> b i p q d", p=128, q=CHUNK_Q)
            zo_v = z_out[:].rearrange("b (i q p) d -> b i p q d", p=128, q=CHUNK_Q)

            paccs = {}

            def slot_of(b, i):
                return (b * NCHUNK + i) % NSLOT

            def sl_of(b, i):
                s0 = slot_of(b, i) * CHUNK_F
                return res[:, s0 : s0 + CHUNK_F]

            def load_chunk(b, i):
                """in-DMA + bf16 shadow + pooling matmuls for chunk i of batch b.

                Chunks 0-3 accumulate into pacc_a, 4-7 into pacc_b; scores are
                linear in the pool sum, so the first half's score contribution
                is flushed mid-stream (partial_scores) and the end-of-batch
                critical chain only covers the second half.
                """
                sl = sl_of(b, i)
                nc.sync.dma_start(
                    out=sl.rearrange("p (q d) -> p q d", q=CHUNK_Q),
                    in_=z_v[b, i],
                )
                pacc = paccs[b]
                first, last = i == 0, i == NCHUNK - 1
                for half in range(2):
                    bft = bfp.tile([128, CHUNK_F // 2], bf16, tag="bfs",
                                   name="bft")
                    nc.scalar.activation(
                        out=bft[:],
                        in_=sl[:, half * (CHUNK_F // 2) : (half + 1) * (CHUNK_F // 2)],
                        func=mybir.ActivationFunctionType.Copy,
                    )
                    for jj in range(4):  # 512-wide columns; d-half h = j % 2
                        j = half * 4 + jj
                        h = j % 2
                        nc.tensor.matmul(
                            pacc[:, h * 512 : (h + 1) * 512],
                            ones_k[:],
                            bft[:, jj * 512 : (jj + 1) * 512],
                            start=(first and j < 2),
                            stop=(last and j >= 6),
                        )

            sc_accs = {}

            def partial_scores(b, sc_first, sc_last):
                """Flush pacc into the scores accumulator."""
                pacc = paccs[b]
                pooled = small.tile([1, 1024], bf16, tag="pooled")
                nc.scalar.activation(
                    out=pooled[:], in_=pacc[:],
                    func=mybir.ActivationFunctionType.Copy, scale=SCALE,
                )
                pt_ps = bpsum.tile([128, 8], f32, tag="pt")
                for c in range(8):
                    nc.tensor.matmul(
                        pt_ps[:, c : c + 1],
                        pooled[:, c * 128 : (c + 1) * 128],
                        one_1[:], start=True, stop=True,
                    )
                pooled_t = small.tile([128, 8], bf16, tag="pooled_t")
                nc.vector.tensor_copy(out=pooled_t[:], in_=pt_ps[:])

                if sc_first:
                    sc_accs[b] = bpsum.tile(
                        [1, 2 * ND], f32, tag="sc", name=f"sc_ps{b}"
                    )
                sc_ps = sc_accs[b]
                for c in range(8):
                    nc.tensor.matmul(
                        sc_ps[:],
                        pooled_t[:, c : c + 1],
                        m_t[:, c * 2 * ND : (c + 1) * 2 * ND],
                        start=(sc_first and c == 0),
                        stop=(sc_last and c == 7),
                    )

            def phase_b(b):
                """scores -> threshold -> r_bc for batch b (second-half flush
                plus the tiny decision chain).

                Returns (r_bc, diff); the mlc sigmoid/DMA is emitted
                separately (phase_b_mlc) so it doesn't sit in the ACT FIFO
                ahead of the next batch's bf16 shadow copies.
                """
                partial_scores(b, sc_first=True, sc_last=True)
                sc_ps = sc_accs[b]

                sc_sb = small.tile([1, 2 * ND], f32, tag="sc_sb")
                nc.vector.tensor_copy(out=sc_sb[:], in_=sc_ps[:])
                sc_v = sc_sb[:].rearrange("p (n s) -> p n s", s=2)
                diff = small.tile([1, ND], f32, tag="diff")
                nc.vector.tensor_sub(diff[:], sc_v[:, :, 1], sc_v[:, :, 0])

                # sigmoid(x) > t  <=>  x > logit(t); scores are ~1e-4 and the
                # threshold is at logit(0.2) = -1.386, so no tie risk.
                ah = small.tile([1, ND], f32, tag="ah")
                nc.vector.tensor_scalar(
                    out=ah[:], in0=diff[:],
                    scalar1=LOGIT_THRESH, scalar2=None,
                    op0=mybir.AluOpType.is_gt,
                )
                # replicate ah along free: [14, 128] with ah_rep[n, p] = ah[n]
                ar_ps = bpsum.tile([ND, 128], f32, tag="sc")
                nc.tensor.matmul(ar_ps[:], ah[:], ones_r[:], start=True, stop=True)
                ah_rep = small.tile([ND, 128], f32, tag="ah_rep")
                nc.vector.tensor_copy(out=ah_rep[:], in_=ar_ps[:])

                # r_bc[p, d] = sum_n ah[n] * M_pres[n, d] (exact fp32),
                # broadcast across partitions in the same matmul.
                r_bc = rbcpool.tile([128, 1024], f32, tag="r_bc")
                for hh in range(2):
                    bc_ps = bpsum.tile([128, 512], f32, tag="r")
                    nc.tensor.matmul(
                        bc_ps[:], ah_rep[:],
                        m_pres[:, hh * 512 : (hh + 1) * 512],
                        start=True, stop=True,
                    )
                    nc.vector.tensor_copy(
                        out=r_bc[:, hh * 512 : (hh + 1) * 512], in_=bc_ps[:]
                    )
                return r_bc, diff

            def phase_b_mlc(b, diff):
                mlc_sb = small.tile([1, ND], f32, tag=f"mlc_sb{b}",
                                    name=f"mlc_sb{b}")
                nc.scalar.activation(
                    out=mlc_sb[:], in_=diff[:],
                    func=mybir.ActivationFunctionType.Sigmoid,
                )
                nc.gpsimd.dma_start(out=mlc_out[b : b + 1, :], in_=mlc_sb[:])

            def add_chunk(b, i, r_bc):
                sl = sl_of(b, i)
                for q in range(CHUNK_Q):
                    nc.vector.tensor_add(
                        sl[:, q * D : (q + 1) * D],
                        sl[:, q * D : (q + 1) * D],
                        r_bc[:],
                    )

            def store_chunk(b, i):
                nc.sync.dma_start(
                    out=zo_v[b, i],
                    in_=sl_of(b, i).rearrange("p (q d) -> p q d", q=CHUNK_Q),
                )

            # ---- emission schedule ----
            # Per-engine FIFOs follow emission order, so each engine's queue
            # is laid out deliberately: the sync queue runs pure-direction
            # stream blocks; phase_b's PE chain is emitted before batch 1's
            # pooling matmuls so the reduction isn't stuck behind them.
            paccs[0] = ppsum.tile([1, 1024], f32, tag="pacc", name="pacc0")
            for i in range(NCHUNK):
                load_chunk(0, i)

            r_bc0, diff0 = phase_b(0)

            # batch 1 chunks 0,1 go to the two spare slots; their loads extend
            # batch 0's read stream and overlap its reduction chain.
            paccs[1] = ppsum.tile([1, 1024], f32, tag="pacc", name="pacc1")
            load_chunk(1, 0)
            load_chunk(1, 1)
            load_chunk(1, 2)
            phase_b_mlc(0, diff0)

            # drain batch 0 (write stream), then the rest of batch 1's loads
            # (read stream); in(1, k+3) reuses the slot freed by out(0, k).
            for k in range(NCHUNK):
                add_chunk(0, k, r_bc0)
                if k < 5:
                    store_chunk(0, k)
            for k in range(3, NCHUNK):
                load_chunk(1, k)
            # the last three stores are deferred: batch 1 doesn't reuse their
            # slots, and at runtime they fill batch 1's reduction bubble.
            store_chunk(0, 5)
            store_chunk(0, 6)
            store_chunk(0, 7)

            r_bc1, diff1 = phase_b(1)
            phase_b_mlc(1, diff1)
            for k in range(NCHUNK):
                add_chunk(1, k, r_bc1)
                store_chunk(1, k)

    nc.finalize()
    return nc


def _get_nc():
    if "nc" not in _CACHE:
        _CACHE["nc"] = _build_nc()
    return _CACHE["nc"]


def kernel(z_fused, M):
    import os

    from concourse.bass_utils import run_bass_kernel_spmd

    global LAST_RESULTS
    nc = _get_nc()
    z_fused = np.ascontiguousarray(z_fused, dtype=np.float32)
    M = np.ascontiguousarray(M, dtype=np.float32)
    in_maps = [
        {"z": z_fused[c * BPC : (c + 1) * BPC], "m": M} for c in range(NCORES)
    ]
    kwargs = {}
    if os.environ.get("KERNEL_TRACE"):
        kwargs["trace"] = True
        if os.environ.get("KERNEL_TMPDIR"):
            kwargs["tmpdir"] = os.environ["KERNEL_TMPDIR"]
    res = run_bass_kernel_spmd(nc, in_maps, list(range(NCORES)), **kwargs)
    LAST_RESULTS = res
    z_out = np.concatenate(
        [res.results[c]["z_out"] for c in range(NCORES)], axis=0
    )
    mlc = np.concatenate([res.results[c]["mlc"] for c in range(NCORES)], axis=0)
    return z_out, mlc
